# revision 36
# baseline (speedup 1.0000x reference)
"""Spatial attention block (GroupNorm + QKV 1x1 + full spatial attention +
out-proj + residual) on 8 Trainium2 NeuronCores.

Sharding: core = (batch b, spatial quarter j). Each core receives its batch
image rotated along the flattened spatial axis by -1024*j, so the SPMD
program always computes attention outputs for "the first 1024 query
positions" of its input. Attention is invariant to a joint rotation of the
K/V spatial axis, and GroupNorm stats are rotation-invariant, so the host
just concatenates the per-core [256, 1024] outputs.

Attention softmax: exp work is split between the Scalar engine (table exp)
and the Vector engine (Schraudolph-style int16 trick writing bf16 bits) so
the two engines each exponentiate one head of the active pair while the
tensor engine runs QK^T for the next chunk / AV for the current one.
"""

import sys

for _p in ("/opt/trn_rl_repo", "/root/.axon_site/_ro/trn_rl_repo"):
    if _p not in sys.path:
        sys.path.insert(0, _p)

import numpy as np

import concourse.bacc as bacc
import concourse.bass as bass
import concourse.tile as tile
from concourse import mybir
from concourse.bass_utils import run_bass_kernel_spmd

F32 = mybir.dt.float32
F32R = mybir.dt.float32r
BF16 = mybir.dt.bfloat16
I16 = mybir.dt.int16
I8 = mybir.dt.int8
F8 = mybir.dt.float8e4
AF = mybir.ActivationFunctionType

B, C, H, W = 2, 256, 64, 64
S = H * W              # 4096 spatial positions
NH = 4                 # heads
HD = C // NH           # 64 head dim
NQ = S // 4            # 1024 query positions per core
NCHUNK = S // 128      # 32 key chunks
NDC = NCHUNK // 2      # 16 double-chunks (256 keys, DoubleRow contraction)
EPS = 1e-5
SCALE = 1.0 / 16.0     # 1/sqrt(C)
VW8 = 80               # per-head stride in the fp8 [V^T | ones] tile (16-aligned)

# Schraudolph fast-exp constants: fp8e4m3 bits of exp(s/16) ~= int8(A*s + B)
# (fit: C=-0.35 centers the piecewise-linear error; +0.5 compensates the DVE
# float->int truncation)
EXP_A = 8.0 / 0.6931471805599453 * SCALE
EXP_B = 56.0 - 0.35 + 0.5
# per-tile engine cost (ns, HW-measured) used to balance exp work
EXP_COST_SCALAR = 1113.0
EXP_COST_VECTOR = 1223.0
DW = 544  # per-den-slot stride in the batched denominator tile


def _build_program():
    nc = bacc.Bacc(None)

    x_d = nc.declare_dram_parameter("x", [C, S], BF16, isOutput=False)
    wqkvT_d = nc.declare_dram_parameter("wqkvT", [C, 3 * C], BF16, isOutput=False)
    woutT_d = nc.declare_dram_parameter("woutT", [NH, HD, C], BF16, isOutput=False)
    gnw_d = nc.declare_dram_parameter("gnw", [2, 128, 1], F32, isOutput=False)
    gnb_d = nc.declare_dram_parameter("gnb", [2, 128, 1], F32, isOutput=False)
    ob_d = nc.declare_dram_parameter("ob", [2, 128, 1], F32, isOutput=False)
    gsel_d = nc.declare_dram_parameter("gsel", [128, 8], F32R, isOutput=False)
    gselT_d = nc.declare_dram_parameter("gselT", [8, 128], F32R, isOutput=False)
    y_d = nc.declare_dram_parameter("y", [C, NQ], F32, isOutput=True)

    with tile.TileContext(nc) as tc, nc.allow_low_precision("fp32r matmul inputs"):
        _emit(nc, tc, x_d, wqkvT_d, woutT_d, gnw_d, gnb_d, ob_d, gsel_d, gselT_d, y_d)
    nc.finalize()
    return nc


def _emit(nc, tc, x_d, wqkvT_d, woutT_d, gnw_d, gnb_d, ob_d, gsel_d, gselT_d, y_d):
    from contextlib import ExitStack

    ctx = ExitStack()
    with ctx:
        # PSUM budget: pp ring 3 x [128,1024]f32 (2 banks each) = 6 banks,
        # po ring 2 x 2KB = 2 banks -> 8 banks exactly.
        persist = ctx.enter_context(tc.tile_pool(name="persist", bufs=1))
        pp = ctx.enter_context(tc.tile_pool(name="pp", bufs=3, space="PSUM"))
        po = ctx.enter_context(tc.tile_pool(name="po", bufs=2, space="PSUM"))

        # ---- persistent SBUF tiles -------------------------------------
        x_sb = [persist.tile([128, S], BF16, tag=f"x{t}", name=f"x{t}") for t in range(2)]
        k_sb = [persist.tile([128, S], BF16, tag=f"k{t}", name=f"k{t}") for t in range(2)]
        q_sb = [persist.tile([128, NQ], BF16, tag=f"q{t}", name=f"q{t}") for t in range(2)]
        # fp8 [V^T | 1] per double-chunk: [128, (chunk-parity, head, 80)]
        vt_sb = [
            persist.tile([128, 2 * NH * VW8], F8, tag=f"vt{d}", name=f"vt{d}")
            for d in range(NDC)
        ]
        attn_sb = [persist.tile([64, NQ], BF16, tag=f"at{h}", name=f"at{h}") for h in range(NH)]
        wq_sb = [persist.tile([128, 3 * C], BF16, tag=f"wq{t}", name=f"wq{t}") for t in range(2)]
        wo_sb = [persist.tile([HD, C], BF16, tag=f"wo{ct}", name=f"wo{ct}") for ct in range(NH)]
        gnw_sb = [persist.tile([128, 1], F32, tag=f"gw{t}", name=f"gw{t}") for t in range(2)]
        gnb_sb = [persist.tile([128, 1], F32, tag=f"gb{t}", name=f"gb{t}") for t in range(2)]
        ob_sb = [persist.tile([128, 1], F32, tag=f"obias{t}", name=f"obias{t}") for t in range(2)]
        gsel_sb = persist.tile([128, 8], F32R, tag="gsel")
        gselT_sb = persist.tile([8, 128], F32R, tag="gselT")
        oacc_sb = [
            persist.tile([128, NQ], F32, tag=f"oacc{t}", name=f"oacc{t}")
            for t in range(2)
        ]
        eps_sb = persist.tile([128, 1], F32, tag="eps")
        nc.vector.memset(eps_sb, EPS)
        # batched softmax denominators: slot idx -> row 32*(idx%4), col-half
        # idx//4. Rows between are untouched garbage (harmless through Ln).
        dens_sb = persist.tile([128, 2 * DW], F32, tag="dens")
        rs_sb = persist.tile([128, 2 * DW], F32R, tag="rs")
        ones97_sb = persist.tile([97, 64], F32R, tag="ones97")

        nc.scalar.activation(
            out=ones97_sb, in_=x_sb[0][0:97, 0:64], func=AF.Copy, scale=0.0, bias=1.0
        )

        for t in range(2):
            for xc in range(4):
                eng = nc.sync if xc % 2 == 0 else nc.gpsimd
                eng.dma_start(
                    out=x_sb[t][:, 1024 * xc : 1024 * (xc + 1)],
                    in_=x_d[128 * t : 128 * (t + 1), 1024 * xc : 1024 * (xc + 1)],
                )
            nc.sync.dma_start(out=wq_sb[t], in_=wqkvT_d[128 * t : 128 * (t + 1), :])
            nc.sync.dma_start(out=gnw_sb[t], in_=gnw_d[t])
            nc.sync.dma_start(out=gnb_sb[t], in_=gnb_d[t])
            nc.sync.dma_start(out=ob_sb[t], in_=ob_d[t])
        for ct in range(NH):
            nc.sync.dma_start(out=wo_sb[ct], in_=woutT_d[ct])
        nc.sync.dma_start(out=gsel_sb, in_=gsel_d[:])
        nc.sync.dma_start(out=gselT_sb, in_=gselT_d[:])
        for oct_ in range(2):
            nc.vector.tensor_scalar(
                out=oacc_sb[oct_],
                in0=x_sb[oct_][:, 0:NQ],
                scalar1=ob_sb[oct_],
                scalar2=None,
                op0=mybir.AluOpType.add,
            )

        # ones columns of the [V^T | ones] tiles (fp8 1.0 = bits 56)
        for d in range(NDC):
            ones_cols = vt_sb[d].bitcast(I8).rearrange(
                "p (two h x) -> p two h x", two=2, h=NH
            )[:, :, :, HD : HD + 1]
            nc.vector.memset(ones_cols, 56)

        warm_n = [0]

        def emit_warm(n, dep=None):
            # keep the PE's HAM activity monitor at full clock through the
            # DMA/GroupNorm phase; `dep` ties a batch to an arriving x slab
            # so the batches spread out instead of all running up front
            d = po.tile([128, 512], F32, tag="po", name=f"warm{warm_n[0]}")
            warm_n[0] += 1
            src = wq_sb[0] if dep is None else dep
            for _ in range(n):
                nc.tensor.matmul(
                    d, src[:, 0:128], src[:, 0:512], start=True, stop=True
                )

        emit_warm(8)
        for t in range(2):
            for xc in range(4):
                emit_warm(4, dep=x_sb[t][:, 1024 * xc : 1024 * xc + 512])

        # ---- GroupNorm -------------------------------------------------
        # per-channel stats via bn_stats (free-dim), then combine the 16
        # channels of each group across partitions with small PE matmuls.
        with tc.tile_pool(name="gnpool", bufs=1) as gnp, tc.tile_pool(
            name="xn", bufs=1
        ) as xnp:
            xn_sb = [xnp.tile([128, S], BF16, tag=f"xn{t}", name=f"xn{t}") for t in range(2)]
            s_t = []
            b_t = []
            g2_l = []
            mr_l = []
            vg2 = gnp.tile([8, 2], F32, tag="vg2")
            rstd2 = gnp.tile([8, 2], F32, tag="rstd2")
            for t in range(2):
                nsub = S // 512
                st6 = gnp.tile([128, nsub, 6], F32, tag=f"st6_{t}")
                for i in range(nsub):
                    nc.vector.bn_stats(
                        out=st6[:, i, :], in_=x_sb[t][:, 512 * i : 512 * (i + 1)]
                    )
                mv = gnp.tile([128, 2], F32, tag=f"mv{t}")
                nc.vector.bn_aggr(out=mv, in_=st6)
                # stats2 = [mean, var + mean^2]  (per channel)
                stats2 = gnp.tile([128, 2], F32R, tag=f"s2_{t}")
                nc.vector.tensor_copy(out=stats2[:, 0:1], in_=mv[:, 0:1])
                nc.vector.tensor_tensor(
                    out=stats2[:, 1:2],
                    in0=mv[:, 0:1],
                    in1=mv[:, 0:1],
                    op=mybir.AluOpType.mult,
                )
                nc.vector.tensor_tensor(
                    out=stats2[:, 1:2],
                    in0=stats2[:, 1:2],
                    in1=mv[:, 1:2],
                    op=mybir.AluOpType.add,
                )
                # group sums: [8, 2] = gsel.T @ stats2, then /16
                pg = pp.tile([8, 2], F32, tag="pp")
                nc.tensor.matmul(pg, (gsel_sb), (stats2), start=True, stop=True)
                g2 = gnp.tile([8, 2], F32, tag=f"g2_{t}")
                nc.scalar.activation(out=g2, in_=pg, func=AF.Copy, scale=1.0 / 16.0)
                # var_g = m2_g - mu_g^2 ; rstd = 1/sqrt(var+eps)
                mr = gnp.tile([8, 2], F32R, tag=f"mr{t}")
                nc.vector.tensor_copy(out=mr[:, 0:1], in_=g2[:, 0:1])
                vg = vg2[:, t : t + 1]
                nc.vector.tensor_tensor(
                    out=vg, in0=g2[:, 0:1], in1=g2[:, 0:1], op=mybir.AluOpType.mult
                )
                nc.vector.tensor_tensor(
                    out=vg, in0=g2[:, 1:2], in1=vg, op=mybir.AluOpType.subtract
                )
                g2_l.append(g2)
                mr_l.append(mr)
                emit_warm(8)
            # rstd = exp(-0.5*ln(var+eps)); both groups' columns share one
            # [8,2] tile so Ln and Exp are single instructions (one table
            # set switch each, and the scheduler cannot interleave them)
            nc.scalar.activation(out=vg2, in_=vg2, func=AF.Ln, bias=eps_sb[0:8])
            nc.scalar.activation(out=rstd2, in_=vg2, func=AF.Exp, scale=-0.5)
            for t in range(2):
                nc.vector.tensor_copy(
                    out=mr_l[t][:, 1:2], in_=rstd2[:, t : t + 1]
                )
            for t in range(2):
                # broadcast (mu, rstd) to the 16 channels of each group
                pb = pp.tile([128, 2], F32, tag="pp")
                nc.tensor.matmul(pb, (gselT_sb), (mr_l[t]), start=True, stop=True)
                # scale = gnw * rstd ; bias = gnb - mu * scale
                sc = gnp.tile([128, 1], F32, tag=f"sc{t}")
                bi = gnp.tile([128, 1], F32, tag=f"bi{t}")
                nc.vector.tensor_tensor(
                    out=sc, in0=gnw_sb[t], in1=pb[:, 1:2], op=mybir.AluOpType.mult
                )
                nc.vector.tensor_tensor(
                    out=bi, in0=pb[:, 0:1], in1=sc, op=mybir.AluOpType.mult
                )
                nc.vector.tensor_tensor(
                    out=bi, in0=gnb_sb[t], in1=bi, op=mybir.AluOpType.subtract
                )
                s_t.append(sc)
                b_t.append(bi)
            for t in range(2):
                nc.vector.tensor_scalar(
                    out=xn_sb[t],
                    in0=x_sb[t],
                    scalar1=s_t[t],
                    scalar2=b_t[t],
                    op0=mybir.AluOpType.mult,
                    op1=mybir.AluOpType.add,
                )

            # ---- projections (inside xn pool scope) --------------------
            # Q first (first attention matmul needs Q + K slab 0 only)
            for t in range(2):
                for sb in range(NQ // 512):
                    ps = pp.tile([128, 512], F32, tag="pp")
                    for kc in range(2):
                        nc.tensor.matmul(
                            ps,
                            (wq_sb[kc][:, 128 * t : 128 * (t + 1)]),
                            (xn_sb[kc][:, 512 * sb : 512 * (sb + 1)]),
                            start=(kc == 0),
                            stop=(kc == 1),
                        )
                    nc.scalar.activation(
                        out=q_sb[t][:, 512 * sb : 512 * (sb + 1)], in_=ps, func=AF.Copy
                    )
            # K: [256 kch, S];  kch tile t holds heads 2t, 2t+1
            for t in range(2):
                for sb in range(S // 512):
                    ps = pp.tile([128, 512], F32, tag="pp")
                    for kc in range(2):
                        nc.tensor.matmul(
                            ps,
                            (wq_sb[kc][:, C + 128 * t : C + 128 * (t + 1)]),
                            (xn_sb[kc][:, 512 * sb : 512 * (sb + 1)]),
                            start=(kc == 0),
                            stop=(kc == 1),
                        )
                    nc.scalar.activation(
                        out=k_sb[t][:, 512 * sb : 512 * (sb + 1)], in_=ps, func=AF.Copy
                    )
            # V^T: [S, 256] per double-chunk (both 128-row chunks side by
            # side in one single-bank PSUM tile), evacuated (fp8) with one
            # strided copy into the per-(parity, head) [V^T | 1] layout.
            for d in range(NDC):
                ps = pp.tile([128, 2 * C], F32, tag="pp")
                for cc in range(2):
                    c = 2 * d + cc
                    for kc in range(2):
                        nc.tensor.matmul(
                            ps[:, C * cc : C * (cc + 1)],
                            (xn_sb[kc][:, 128 * c : 128 * (c + 1)]),
                            (wq_sb[kc][:, 2 * C : 3 * C]),
                            start=(kc == 0),
                            stop=(kc == 1),
                        )
                vdst = vt_sb[d].rearrange(
                    "p (two h x) -> p two h x", two=2, h=NH
                )[:, :, :, 0:HD]
                nc.vector.tensor_copy(
                    out=vdst, in_=ps.rearrange("p (two h x) -> p two h x", two=2, h=NH)
                )

        # ---- attention -------------------------------------------------
        # per (head-pair, 512-query half), in double-chunks of 256 keys:
        # QK writes a [128, 1024] 2-bank score tile per head (one bank per
        # 128-key chunk), ONE exp instruction covers both chunks, and the AV
        # is a single fp8 DoubleRow matmul contracting all 256 keys.
        with tc.tile_pool(name="epool", bufs=3) as ep, tc.tile_pool(
            name="npool", bufs=1
        ) as np_pool:
            # oh[h][nb], rs[h][nb]
            oh_l = [[None] * 2 for _ in range(NH)]
            # accumulated ns on (scalar, vector); vector seeded for the
            # V-evacuation copies that trail into early attention
            exp_cost = [0.0, 5000.0]

            def den_slot(h, nb):
                idx = 2 * h + nb
                return 32 * (idx % 4), DW * (idx // 4)

            def head_evac(h, nb, po_h):
                # evacuate the accumulator to SBUF right away so the PSUM
                # slot frees for the next (pair, half)'s AV matmuls.
                # On VectorE: a ScalarE Copy here would switch the activation
                # table set away from Exp mid-attention (2.6us round trip).
                oh = np_pool.tile([65, 512], F32, tag=f"oh{h}_{nb}", name=f"oh{h}_{nb}")
                nc.vector.tensor_copy(out=oh, in_=po_h)
                # stage the denominator row into its 32-aligned batch slot
                row, col = den_slot(h, nb)
                nc.vector.tensor_copy(
                    out=dens_sb[row : row + 1, col : col + 512], in_=po_h[64:65, :]
                )
                exp_cost[1] += 1500.0
                oh_l[h][nb] = oh

            def emit_qk(pair, d, nb):
                # scores for double-chunk d: per head a [128, 1024] tile,
                # chunk 2d in cols 0:512, chunk 2d+1 in cols 512:1024.
                # The pair's two QK matmuls per chunk target disjoint PE row
                # groups (rows 0-63 / 64-127) and run concurrently.
                ps2 = [
                    pp.tile([128, 1024], F32, tag="pp", name=f"s{pair}_{d}_{nb}_{i}")
                    for i in range(2)
                ]
                for cc in range(2):
                    c = 2 * d + cc
                    for i in range(2):
                        row = i * 64
                        nc.tensor.matmul(
                            ps2[i][:, 512 * cc : 512 * (cc + 1)],
                            (k_sb[pair][row : row + 64, 128 * c : 128 * (c + 1)]),
                            (q_sb[pair][row : row + 64, 512 * nb : 512 * (nb + 1)]),
                            start=True,
                            stop=True,
                        )
                return ps2

            last_e = [None]

            def emit_av(pair, d, nb, ps2, po_pair):
                # exp of both chunks in one instruction per head, balanced
                # between ScalarE (table exp -> fp8) and VectorE (Schraudolph
                # int8 -> fp8 bits); then one DoubleRow AV matmul per head.
                for i in range(2):
                    h = 2 * pair + i
                    e_t = ep.tile(
                        [128, 1024], F8, tag=f"e{i}", name=f"e{pair}_{d}_{nb}_{i}"
                    )
                    if exp_cost[0] + EXP_COST_SCALAR <= exp_cost[1] + EXP_COST_VECTOR:
                        exp_cost[0] += EXP_COST_SCALAR
                        nc.scalar.activation(
                            out=e_t, in_=ps2[i], func=AF.Exp, scale=SCALE
                        )
                    else:
                        exp_cost[1] += EXP_COST_VECTOR
                        nc.vector.tensor_scalar(
                            out=e_t.bitcast(I8),
                            in0=ps2[i],
                            scalar1=EXP_A,
                            scalar2=EXP_B,
                            op0=mybir.AluOpType.mult,
                            op1=mybir.AluOpType.add,
                        )
                    last_e[0] = e_t
                    nc.tensor.matmul(
                        po_pair[i],
                        vt_sb[d].rearrange("p (two h x) -> p two h x", two=2, h=NH)[
                            :, :, h, 0 : HD + 1
                        ],
                        e_t.rearrange("p (two n) -> p two n", two=2),
                        start=(d == 0),
                        stop=(d == NDC - 1),
                        perf_mode=mybir.MatmulPerfMode.DoubleRow,
                    )

            def emit_norm(h, nb):
                # attn[h] = oh[0:64] * (1/denominator) broadcast via ones-matmul
                row, col = den_slot(h, nb)
                pb = pp.tile([64, 512], F32, tag="pp", name=f"pb{h}_{nb}")
                nc.tensor.matmul(
                    pb,
                    ones97_sb[row : row + 1, :],
                    rs_sb[row : row + 1, col : col + 512],
                    start=True,
                    stop=True,
                    tile_position=(row, 0),
                )
                nc.vector.tensor_tensor(
                    out=attn_sb[h][:, 512 * nb : 512 * (nb + 1)],
                    in0=oh_l[h][nb][0:64, :],
                    in1=pb,
                    op=mybir.AluOpType.mult,
                )

            # software-pipelined attention: the PE queue always has QK(d+1)
            # enqueued before AV(d), so the array never stalls on exp(d).
            for pair in range(2):
                for nb in range(2):
                    po_pair = [
                        po.tile([65, 512], F32, tag="po", name=f"po{pair}_{nb}_{i}")
                        for i in range(2)
                    ]
                    ps_cur = emit_qk(pair, 0, nb)
                    for d in range(NDC):
                        ps_next = emit_qk(pair, d + 1, nb) if d + 1 < NDC else None
                        emit_av(pair, d, nb, ps_cur, po_pair)
                        ps_cur = ps_next
                    for i in range(2):
                        head_evac(2 * pair + i, nb, po_pair[i])

            # Gate the batched denominator tile behind the final exp by
            # writing its pad region from the last e tile: the reciprocal
            # Ln/Exp below then has a hard data dependency on the end of
            # attention, so the scheduler cannot hoist it (and its activation
            # table switches) into the attention phase.
            nc.vector.tensor_copy(
                out=dens_sb[64:65, 2 * DW - 8 : 2 * DW], in_=last_e[0][64:65, 0:8]
            )
            # 1/den via exp(-ln(den)): one Ln + one Exp for all 8 den rows
            # (rows between the 32-aligned slots hold garbage; Ln of garbage
            # is never read downstream)
            nc.scalar.activation(out=dens_sb[0:97, :], in_=dens_sb[0:97, :], func=AF.Ln)
            nc.scalar.activation(
                out=rs_sb[0:97, :], in_=dens_sb[0:97, :], func=AF.Exp, scale=-1.0
            )

            for nb in range(2):
                pf_acc = [
                    po.tile([128, 512], F32, tag="po", name=f"pfa{o}_{nb}")
                    for o in range(2)
                ]
                for h in range(NH):
                    emit_norm(h, nb)
                    for oct_ in range(2):
                        nc.tensor.matmul(
                            pf_acc[oct_],
                            (wo_sb[h][:, 128 * oct_ : 128 * (oct_ + 1)]),
                            attn_sb[h][:, 512 * nb : 512 * (nb + 1)],
                            start=(h == 0),
                            stop=(h == NH - 1),
                        )
                for oct_ in range(2):
                    nc.vector.tensor_tensor(
                        out=oacc_sb[oct_][:, 512 * nb : 512 * (nb + 1)],
                        in0=oacc_sb[oct_][:, 512 * nb : 512 * (nb + 1)],
                        in1=pf_acc[oct_],
                        op=mybir.AluOpType.add,
                    )
                    # store each quarter as soon as it is final
                    eng = nc.sync if oct_ == 0 else nc.gpsimd
                    eng.dma_start(
                        out=y_d[128 * oct_ : 128 * (oct_ + 1), 512 * nb : 512 * (nb + 1)],
                        in_=oacc_sb[oct_][:, 512 * nb : 512 * (nb + 1)],
                    )


_PROGRAM = None


def _get_program():
    global _PROGRAM
    if _PROGRAM is None:
        _PROGRAM = _build_program()
    return _PROGRAM


def _prep_inputs(input, gn_weight, gn_bias, qkv_weight, out_weight, out_bias):
    input = np.asarray(input, dtype=np.float32).reshape(B, C, S)
    gn_weight = np.asarray(gn_weight, dtype=np.float32)
    gn_bias = np.asarray(gn_bias, dtype=np.float32)
    qkv_weight = np.asarray(qkv_weight, dtype=np.float32)
    out_weight = np.asarray(out_weight, dtype=np.float32)
    out_bias = np.asarray(out_bias, dtype=np.float32)

    # reference splits qkv head-major: rows 192h..192h+192 = [q|k|v] of head h.
    # device layout wants cols [Q heads 0..3 | K heads 0..3 | V heads 0..3].
    perm = np.concatenate(
        [
            np.arange(192 * h + 64 * part, 192 * h + 64 * (part + 1))
            for part in range(3)
            for h in range(NH)
        ]
    )
    import ml_dtypes

    wqkvT = np.ascontiguousarray(qkv_weight.T[:, perm]).astype(ml_dtypes.bfloat16)
    # reorder: per head h, q rows h*64..h*64+64 map to wqkvT cols as-is
    woutT = np.ascontiguousarray(out_weight.T.reshape(NH, HD, C)).astype(
        ml_dtypes.bfloat16
    )
    gnw = np.ascontiguousarray(gn_weight.reshape(2, 128, 1))
    gnb = np.ascontiguousarray(gn_bias.reshape(2, 128, 1))
    ob = np.ascontiguousarray(out_bias.reshape(2, 128, 1))
    gsel = np.zeros((128, 8), np.float32)
    for p in range(128):
        gsel[p, p // 16] = 1.0
    gselT = np.ascontiguousarray(gsel.T)

    in_maps = []
    for core in range(8):
        b, j = core // 4, core % 4
        xrot = np.roll(input[b], -NQ * j, axis=1)
        in_maps.append(
            {
                "x": np.ascontiguousarray(xrot).astype(ml_dtypes.bfloat16),
                "wqkvT": wqkvT,
                "woutT": woutT,
                "gnw": gnw,
                "gnb": gnb,
                "ob": ob,
                "gsel": gsel,
                "gselT": gselT,
            }
        )
    return in_maps


def kernel(input, gn_weight, gn_bias, qkv_weight, out_weight, out_bias, _trace=False):
    nc = _get_program()
    in_maps = _prep_inputs(
        input, gn_weight, gn_bias, qkv_weight, out_weight, out_bias
    )
    kw = {}
    if _trace:
        kw = {"trace": True, "tmpdir": "/tmp/attn_trace"}
    res = run_bass_kernel_spmd(nc, in_maps, list(range(8)), **kw)
    out = np.empty((B, C, S), np.float32)
    for core in range(8):
        b, j = core // 4, core % 4
        out[b, :, NQ * j : NQ * (j + 1)] = res.results[core]["y"]
    out = out.reshape(B, C, H, W)
    if _trace:
        return out, res
    return out
